# revision 21
# baseline (speedup 1.0000x reference)
"""Trainium2 Bass kernel for nn_DecoderLayer (dense transformer decoder layer).

Sharding: data-parallel over batch (16 batches -> 8 cores x 2 each). Each core
runs the full decoder layer on its batch slice; no collectives.

v2: bf16 matmul pipeline. All PE matmuls take bf16 operands (fp32 PSUM
accumulation) -> fast weight load (FWL) halves LDWEIGHTS cost vs fp32r, DVE
element-wise ops on bf16 run at 2x, and SBUF footprint halves. On-chip
transposes (weights W -> W^T, activations token-major -> feature-major) go
through the DMA XBAR transpose (2-byte dtype, 14ns per 16x128 tile) instead of
PE transpose-mode matmuls, freeing the Tensor engine for real GEMM work; the
fp32->bf16 casts feeding those transposes run on the scalar (ACT) engine which
has headroom. XBAR-transposed destinations use block-major layouts
([128, blk, C, 128]) so every DMA-transpose destination slice is contiguous
per partition (hardware requirement).

Layout: activations are feature-major ("xT": [feature partitions, token free])
so every linear is psum = W^T.T @ xT. Attention uses transposed scores
S^T = K_hT.T @ Q_hT ([j partitions, i free]), exp without max-subtraction
(scores are bounded), and a ones-column appended to V so the softmax
denominator comes out of the same PV matmul. LayerNorm runs feature-major with
partition sums via ones-vector matmuls and per-token broadcast via K=1
matmuls. Cross-attention streams enc_mem in 512-token chunks per batch so
K/V-projection matmuls run at N=512.
"""
import sys
import numpy as np

sys.path.insert(0, '/opt/trn_rl_repo')

import concourse.bass as bass  # noqa: E402
import concourse.tile as tile  # noqa: E402
from concourse import bacc, mybir  # noqa: E402
from concourse.bass_utils import run_bass_kernel_spmd  # noqa: E402
from concourse.masks import make_identity  # noqa: E402
from contextlib import ExitStack  # noqa: E402

F32 = mybir.dt.float32
BF16 = mybir.dt.bfloat16
AF = mybir.ActivationFunctionType

EPS = 1e-5
N_CORES = 8


def build_decoder(nc, tc, ctx, B_loc, NQ, S, W, NH, MLP, JC=512, suffix=""):
    HD = W // NH
    assert HD == 64 and NQ % 128 == 0 and W % 512 == 0 and JC % 128 == 0
    T = B_loc * NQ          # decoder tokens per core
    TC = T // 128
    WC = W // 128
    MC = MLP // 128
    NJC = S // JC           # enc chunks per batch
    JSC = JC // 128
    NQC = NQ // 128
    SCALE = float(W) ** -0.5
    HPC = 128 // HD         # heads per feature chunk (2)

    dram = {}
    for name, shape in [
        ('query', [B_loc, NQ, W]), ('enc_mem', [B_loc, S, W]),
        ('out_pos_enc', [B_loc, NQ, W]),
        ('sa_wq', [W, W]), ('sa_wk', [W, W]), ('sa_wv', [W, W]), ('sa_wo', [W, W]),
        ('ca_wq', [W, W]), ('ca_wk', [W, W]), ('ca_wv', [W, W]), ('ca_wo', [W, W]),
        ('ffn_w1', [MLP, W]), ('ffn_b1', [MLP]), ('ffn_w2', [W, MLP]), ('ffn_b2', [W]),
        ('ln1_g', [W]), ('ln1_b', [W]), ('ln2_g', [W]), ('ln2_b', [W]),
        ('ln3_g', [W]), ('ln3_b', [W]),
    ]:
        if suffix:
            dram[name] = build_decoder._dram_cache[name]
        else:
            dram[name] = nc.dram_tensor(name, shape, F32, kind="ExternalInput")
    build_decoder._dram_cache = dict(dram)
    out_d = nc.dram_tensor("out" + suffix, [B_loc, NQ, W], F32,
                           kind="ExternalOutput")

    q_flat = dram['query'].rearrange("b n w -> (b n) w")
    pe_flat = dram['out_pos_enc'].rearrange("b n w -> (b n) w")
    m_flat = dram['enc_mem'].rearrange("b s w -> (b s) w")
    out_flat = out_d.rearrange("b n w -> (b n) w")

    # ---------------- global pools ----------------
    consts = ctx.enter_context(tc.tile_pool(name="consts", bufs=1))
    persist = ctx.enter_context(tc.tile_pool(name="persist", bufs=1))
    scratch = ctx.enter_context(tc.tile_pool(name="scratch", bufs=2))
    tp_ps = ctx.enter_context(tc.tile_pool(name="tp_ps", bufs=2, space="PSUM"))
    mm_ps = ctx.enter_context(tc.tile_pool(name="mm_ps", bufs=2, space="PSUM"))
    sc_ps = ctx.enter_context(tc.tile_pool(name="sc_ps", bufs=2, space="PSUM"))
    pv_ps = ctx.enter_context(tc.tile_pool(name="pv_ps", bufs=2, space="PSUM"))

    ident = consts.tile([128, 128], F32, tag="ident")
    make_identity(nc, ident[:])
    ones_f = consts.tile([128, 128], F32, tag="ones_f")
    nc.gpsimd.memset(ones_f[:], 1.0)
    ones_b = consts.tile([128, 128], BF16, tag="ones_b")
    nc.vector.tensor_copy(ones_b[:], ones_f[:])
    eps_t = consts.tile([1, 1], F32, tag="eps")
    nc.gpsimd.memset(eps_t[:], EPS)

    def load_col(name, n):
        """[n] param vector -> [128, n/128] per-partition columns (fp32).

        Contiguous [n/128, 128] landing + one PE transpose."""
        nch = n // 128
        land = scratch.tile([128, 128], F32, tag="colland", bufs=2,
                            name=name + "_land")
        nc.sync.dma_start(land[0:nch, :],
                          dram[name].rearrange("(c p) -> c p", p=128))
        pt = tp_ps.tile([128, 512], F32, tag="tp", name="pt_col")
        nc.tensor.transpose(pt[:, 0:128], land[:, 0:128], ident[:])
        t = consts.tile([128, nch], F32, tag=name, name=name + "_col")
        nc.vector.tensor_copy(t[:], pt[:, 0:nch])
        return t
    cols = {k: load_col(k, W) for k in
            ['ln1_g', 'ln1_b', 'ln2_g', 'ln2_b', 'ln3_g', 'ln3_b', 'ffn_b2']}
    b1_col = load_col('ffn_b1', MLP)

    # ---------------- helpers ----------------
    def load_wT(pool, tag, w_dram, O, I, name, bufs=1):
        """Stream W [O, I] fp32 from DRAM -> bf16 W^T tile, block-major
        [128, O/128, I/128, 128]: wt[p, ob, c, n] = W[ob*128+n, c*128+p].

        DMA fp32 rows -> ACT cast bf16 -> XBAR DMA-transpose (dst slice
        wt[:, ob] is contiguous per partition)."""
        wt = pool.tile([128, O // 128, I // 128, 128], BF16, tag=tag,
                       name=name, bufs=bufs)
        for ob in range(O // 128):
            land = scratch.tile([128, I], F32, tag="wland", bufs=2,
                                name="w_land")
            nc.sync.dma_start(land[:, 0:I], w_dram[ob * 128:(ob + 1) * 128, :])
            cst = scratch.tile([128, I], BF16, tag="wcast", bufs=2,
                               name="w_cast")
            nc.scalar.copy(cst[:, 0:I], land[:, 0:I])
            nc.sync.dma_start_transpose(wt[:, ob], cst[:, 0:I])
        return wt

    def gemm(psum, wt, ob, rhs, ICn):
        """psum[oc 128, N] += sum_ic wt[:, ob, ic, :].T @ rhs(ic)."""
        for ic in range(ICn):
            nc.tensor.matmul(psum, wt[:, ob, ic, :], rhs(ic),
                             start=(ic == 0), stop=(ic == ICn - 1))

    def layernorm(x_fn, n_chunks, N, g_col, b_col, out_fn):
        """Feature-major LN over the partition (feature) dim."""
        ps_s = tp_ps.tile([1, N], F32, tag="tp", name="ps_s")
        for ic in range(n_chunks):
            nc.tensor.matmul(ps_s[0:1, :], ones_b[:, 0:1], x_fn(ic),
                             start=(ic == 0), stop=(ic == n_chunks - 1))
        ps_q = tp_ps.tile([1, N], F32, tag="tp", name="ps_q")
        for ic in range(n_chunks):
            sq = scratch.tile([128, N], BF16, tag="sq", name="sq",
                              padded_shape=[128, 512])
            nc.vector.tensor_mul(sq[:, 0:N], x_fn(ic), x_fn(ic))
            nc.tensor.matmul(ps_q[0:1, :], ones_b[:, 0:1], sq[:, 0:N],
                             start=(ic == 0), stop=(ic == n_chunks - 1))
        inv_w = 1.0 / (n_chunks * 128)
        mu = scratch.tile([1, N], BF16, tag="st_mu", bufs=1, name="mu")
        nc.scalar.activation(mu[0:1, :], ps_s[0:1, :], AF.Copy, scale=inv_w)
        ex2 = scratch.tile([1, N], F32, tag="st_e", bufs=1, name="ex2")
        nc.scalar.activation(ex2[0:1, :], ps_q[0:1, :], AF.Copy, scale=inv_w)
        mu2 = scratch.tile([1, N], F32, tag="st_x", bufs=1, name="mu2")
        nc.vector.tensor_mul(mu2[0:1, :], mu[0:1, :], mu[0:1, :])
        var = scratch.tile([1, N], F32, tag="st_v", bufs=1, name="var")
        nc.vector.tensor_sub(var[0:1, :], ex2[0:1, :], mu2[0:1, :])
        sd = scratch.tile([1, N], F32, tag="st_x", bufs=1, name="sd")
        nc.scalar.activation(sd[0:1, :], var[0:1, :], AF.Sqrt,
                             bias=eps_t[0:1, 0:1])
        rstd = scratch.tile([1, N], BF16, tag="st_r", bufs=1, name="rstd")
        nc.vector.reciprocal(rstd[0:1, :], sd[0:1, :])
        ps_mu = tp_ps.tile([128, N], F32, tag="tp", name="ps_mu")
        nc.tensor.matmul(ps_mu[:, 0:N], ones_b[0:1, :], mu[0:1, :])
        ps_rs = tp_ps.tile([128, N], F32, tag="tp", name="ps_rs")
        nc.tensor.matmul(ps_rs[:, 0:N], ones_b[0:1, :], rstd[0:1, :])
        for ic in range(n_chunks):
            xm = scratch.tile([128, N], F32, tag="xm", name="xm",
                              padded_shape=[128, 512])
            nc.vector.tensor_sub(xm[:, 0:N], x_fn(ic), ps_mu[:, 0:N])
            nc.vector.tensor_mul(xm[:, 0:N], xm[:, 0:N], ps_rs[:, 0:N])
            nc.scalar.activation(out_fn(ic), xm[:, 0:N], AF.Identity,
                                 bias=b_col[:, ic:ic + 1],
                                 scale=g_col[:, ic:ic + 1])

    def normalize_head(h, src, oT, col):
        """oT head slice = src[0:HD] / src[HD] (softmax sums row)."""
        off = (h % HPC) * HD
        fc = h // HPC
        rec = scratch.tile([1, NQ], BF16, tag="rec", bufs=2, name="rec")
        nc.vector.reciprocal(rec[0:1, :], src[HD:HD + 1, :])
        ps_b = mm_ps.tile([HD, NQ], F32, tag="mm", name="ps_bc")
        nc.tensor.matmul(ps_b[0:HD, :], ones_b[0:1, 0:HD], rec[0:1, :])
        nc.vector.tensor_mul(oT[off:off + HD, fc, col:col + NQ], src[0:HD, :],
                             ps_b[0:HD, :])

    def load_tokmaj(pool, flat, t0, nrows, name, bufs=2):
        """DMA [nrows, W] fp32 rows t0..t0+nrows -> cast -> bf16 [128, r, W]."""
        rc = nrows // 128
        land = pool.tile([128, rc, W], F32, tag=name + "_l", bufs=bufs,
                         name=name + "_land")
        nc.sync.dma_start(
            land[:], flat[t0:t0 + nrows, :].rearrange("(c p) w -> p c w",
                                                      p=128))
        cst = pool.tile([128, rc, W], BF16, tag=name + "_c", bufs=bufs,
                        name=name + "_cast")
        nc.scalar.copy(cst[:], land[:])
        return cst

    def xpose_blocks(dstT, cst, blk0, nblk):
        """XBAR-transpose bf16 token-major [128, nblk, W] into block-major
        dstT[:, blk0+i] ([128, WC, 128] each, contiguous per partition)."""
        for i in range(nblk):
            nc.sync.dma_start_transpose(dstT[:, blk0 + i], cst[:, i])

    def attention(b, q2T, k2T, vext, first, n_js, acc, oT=None):
        """One key/value chunk of attention for all heads, batch b.

        k2T [128, WC, n_js*128] bf16; vext [128, n_js, NH, HD+1] bf16;
        acc [HD+1, NH, NQ] f32 accumulators (PV partials + softmax sums),
        or None to write normalized output straight into oT (single chunk).
        Heads are emitted in pairs so consecutive S^T matmuls use disjoint
        PE row groups (offset 0 / 64) and can overlap in the array."""
        def head_scores(h):
            off = (h % HPC) * HD
            fc = h // HPC
            e = scratch.tile([128, n_js, NQ], BF16, tag="exp", bufs=3,
                             name="e", padded_shape=[128, 4, NQ])
            for half in range((n_js + 1) // 2):
                js0 = half * 2
                nsub = min(2, n_js - js0)
                ps_s = sc_ps.tile([128, 2, NQ], F32, tag="sc", name="ps_sc")
                for sub in range(nsub):
                    js = js0 + sub
                    nc.tensor.matmul(
                        ps_s[:, sub, :],
                        k2T[off:off + HD, fc, js * 128:(js + 1) * 128],
                        q2T[off:off + HD, fc, b * NQ:(b + 1) * NQ])
                nc.scalar.activation(e[:, js0:js0 + nsub, :],
                                     ps_s[:, 0:nsub, :], AF.Exp, scale=SCALE)
            return e

        for hp in range(NH // 2):
            e0 = head_scores(2 * hp)
            e1 = head_scores(2 * hp + 1)
            # both heads' PV groups share one PSUM bank (col halves);
            # one DVE accumulate for the pair
            ps_o = pv_ps.tile([HD + 1, 2, NQ], F32, tag="pv", name="ps_pv2")
            for sub, e in ((0, e0), (1, e1)):
                h = 2 * hp + sub
                for js in range(n_js):
                    nc.tensor.matmul(ps_o[0:HD + 1, sub, :],
                                     vext[:, js, h, :], e[:, js, :],
                                     start=(js == 0), stop=(js == n_js - 1))
            if acc is None:
                pv_sb = scratch.tile([HD + 1, 2, NQ], F32, tag="pvsb", bufs=2,
                                     name="pv_sb")
                nc.vector.tensor_copy(pv_sb[:], ps_o[:])
                normalize_head(2 * hp, pv_sb[:, 0, :], oT, 0)
                normalize_head(2 * hp + 1, pv_sb[:, 1, :], oT, 0)
            elif first:
                nc.vector.tensor_copy(acc[0:HD + 1, 2 * hp:2 * hp + 2, :],
                                      ps_o[0:HD + 1, :, :])
            else:
                nc.vector.tensor_add(acc[0:HD + 1, 2 * hp:2 * hp + 2, :],
                                     acc[0:HD + 1, 2 * hp:2 * hp + 2, :],
                                     ps_o[0:HD + 1, :, :])

    def attn_normalize(acc, oT, col):
        for h in range(NH):
            normalize_head(h, acc[:, h, :], oT, col)

    # ================= P0: inputs + self-attention =================
    x1T = persist.tile([128, WC, T], BF16, tag="x1T", name="x1T")
    peT, peT_free = tc.tile([128, TC, WC, 128], BF16, name="peT")
    with nc.named_scope("sa"), \
         tc.tile_pool(name="sa_w", bufs=1) as sa_w, \
         tc.tile_pool(name="sa", bufs=1) as sa:
        qT = sa.tile([128, TC, WC, 128], BF16, tag="qT", name="qT")
        qkT = sa.tile([128, TC, WC, 128], BF16, tag="qkT", name="qkT")
        for b in range(B_loc):
            qc = load_tokmaj(sa, q_flat, b * NQ, NQ, "q", bufs=1)
            pc = load_tokmaj(sa, pe_flat, b * NQ, NQ, "pe", bufs=1)
            xpose_blocks(qT, qc, b * NQC, NQC)
            xpose_blocks(peT, pc, b * NQC, NQC)
        # qkT = qT + peT feature-major (bf16 DVE, 2x rate)
        nc.vector.tensor_add(qkT[:].rearrange("p t c n -> p (t c n)"),
                             qT[:].rearrange("p t c n -> p (t c n)"),
                             peT[:].rearrange("p t c n -> p (t c n)"))

        def qk_rhs(ic):
            return qkT[:, :, ic, :]

        wqt = load_wT(sa_w, "wtA", dram['sa_wq'], W, W, "sa_wq_t", bufs=2)
        qsaT = sa.tile([128, WC, T], BF16, tag="big", bufs=3, name="qsaT")
        for ob in range(WC):
            ps = mm_ps.tile([128, T], F32, tag="mm", name="ps_q")
            gemm(ps[:, 0:T], wqt, ob, qk_rhs, WC)
            nc.vector.tensor_copy(qsaT[:, ob, :], ps[:, 0:T])
        wkt = load_wT(sa_w, "wtA", dram['sa_wk'], W, W, "sa_wk_t", bufs=2)
        ksaT = sa.tile([128, WC, T], BF16, tag="big", bufs=3, name="ksaT")
        for ob in range(WC):
            ps = mm_ps.tile([128, T], F32, tag="mm", name="ps_k")
            gemm(ps[:, 0:T], wkt, ob, qk_rhs, WC)
            nc.vector.tensor_copy(ksaT[:, ob, :], ps[:, 0:T])
        wvt = load_wT(sa_w, "wtA", dram['sa_wv'], W, W, "sa_wv_t", bufs=2)
        vext_all = sa.tile([128, TC, NH, HD + 1], BF16, tag="vext",
                           name="vext_sa")
        for tcx in range(TC):
            for oh in range(W // 512):
                ps = mm_ps.tile([128, 512], F32, tag="mm", name="ps_v")
                for ic in range(WC):
                    nc.tensor.matmul(
                        ps[:, 0:512],
                        qT[:, tcx, ic, :],
                        wvt[:, oh * 4:(oh + 1) * 4, ic, :],
                        start=(ic == 0), stop=(ic == WC - 1))
                nh0 = oh * (512 // HD)
                nc.vector.tensor_copy(
                    vext_all[:, tcx, nh0:nh0 + 512 // HD, 0:HD],
                    ps[:, 0:512].rearrange("p (h d) -> p h d", d=HD))
            nc.vector.tensor_copy(vext_all[:, tcx, :, HD], ones_f[:, 0:NH])
        wot = load_wT(sa_w, "wtA", dram['sa_wo'], W, W, "sa_wo_t", bufs=2)

        osaT = sa.tile([128, WC, NQ], BF16, tag="osaT", name="osaT")
        x1pre = sa.tile([128, WC, NQ], BF16, tag="x1pre", name="x1pre")
        for b in range(B_loc):
            attention(b, qsaT, ksaT[:, :, b * NQ:(b + 1) * NQ],
                      vext_all[:, b * NQC:(b + 1) * NQC, :, :],
                      True, NQC, None, oT=osaT)
            for ob in range(WC):
                ps = mm_ps.tile([128, NQ], F32, tag="mm", name="ps_o")
                gemm(ps[:, 0:NQ], wot, ob, lambda ic: osaT[:, ic, :], WC)
                nc.vector.tensor_add(
                    x1pre[:, ob, :].rearrange("p (c n) -> p c n", n=128),
                    ps[:, 0:NQ].rearrange("p (c n) -> p c n", n=128),
                    qT[:, b * NQC:(b + 1) * NQC, ob, :])
            layernorm(lambda ic: x1pre[:, ic, :], WC, NQ,
                      cols['ln1_g'], cols['ln1_b'],
                      lambda ic: x1T[:, ic, b * NQ:(b + 1) * NQ])

    # ================= cross-attention =================
    q2T = persist.tile([128, WC, T], BF16, tag="q2T", name="q2T")
    with nc.named_scope("ca_q"), tc.tile_pool(name="ca_early", bufs=1) as cae:
        x1pT = cae.tile([128, WC, T], BF16, tag="x1pT", name="x1pT")
        for fc in range(WC):
            nc.vector.tensor_add(
                x1pT[:, fc, :].rearrange("p (t n) -> p t n", n=128),
                x1T[:, fc, :].rearrange("p (t n) -> p t n", n=128),
                peT[:, :, fc, :])
        wqt2 = load_wT(cae, "wtQ", dram['ca_wq'], W, W, "ca_wq_t")
        for ob in range(WC):
            ps = mm_ps.tile([128, T], F32, tag="mm", name="ps_q2")
            gemm(ps[:, 0:T], wqt2, ob, lambda ic: x1pT[:, ic, :], WC)
            nc.vector.tensor_copy(q2T[:, ob, :], ps[:, 0:T])
    peT_free()

    with nc.named_scope("ca"), \
         tc.tile_pool(name="ca_w", bufs=1) as ca_w, \
         tc.tile_pool(name="ca", bufs=1) as ca:
        wvt2 = load_wT(ca_w, "wtV", dram['ca_wv'], W, W, "ca_wv_t")
        wkt2 = load_wT(ca_w, "wtA", dram['ca_wk'], W, W, "ca_wk_t", bufs=2)

        ocaT = ca.tile([128, WC, T], BF16, tag="ocaT", name="ocaT")
        with tc.tile_pool(name="ca_acc", bufs=1) as cacc, \
             tc.tile_pool(name="ca_jc", bufs=1) as cjc:
            for b in range(B_loc):
                acc = cacc.tile([HD + 1, NH, NQ], F32, tag="acc",
                                name="acc_ca")
                for jc in range(NJC):
                    tok0 = b * S + jc * JC
                    # land + cast in 1-block pieces, XBAR-transpose each
                    mT = cjc.tile([128, JSC, WC, 128], BF16, tag="mT",
                                  bufs=2, name="mT")
                    for sj in range(JSC):
                        mc = load_tokmaj(cjc, m_flat, tok0 + sj * 128, 128,
                                         "m", bufs=3)
                        xpose_blocks(mT, mc, sj, 1)
                    k2T = cjc.tile([128, WC, JC], BF16, tag="k2T", bufs=2,
                                   name="k2T")
                    for ob in range(WC):
                        ps = mm_ps.tile([128, JC], F32, tag="mm",
                                        name="ps_k2")
                        gemm(ps[:, 0:JC], wkt2, ob,
                             lambda ic: mT[:, :, ic, :], WC)
                        nc.vector.tensor_copy(k2T[:, ob, :], ps[:, 0:JC])
                    vext = cjc.tile([128, JSC, NH, HD + 1], BF16, tag="vext",
                                    bufs=2, name="vext_ca")
                    for sj in range(JSC):
                        for oh in range(W // 512):
                            ps = mm_ps.tile([128, 512], F32, tag="mm",
                                            name="ps_v2")
                            for ic in range(WC):
                                nc.tensor.matmul(
                                    ps[:, 0:512],
                                    mT[:, sj, ic, :],
                                    wvt2[:, oh * 4:(oh + 1) * 4, ic, :],
                                    start=(ic == 0), stop=(ic == WC - 1))
                            nh0 = oh * (512 // HD)
                            nc.vector.tensor_copy(
                                vext[:, sj, nh0:nh0 + 512 // HD, 0:HD],
                                ps[:, 0:512].rearrange("p (h d) -> p h d",
                                                       d=HD))
                        nc.vector.tensor_copy(vext[:, sj, :, HD],
                                              ones_f[:, 0:NH])
                    attention(b, q2T, k2T, vext, jc == 0, JSC, acc)
                attn_normalize(acc, ocaT, b * NQ)

        wot2 = load_wT(ca_w, "wtA", dram['ca_wo'], W, W, "ca_wo_t", bufs=2)
        with tc.tile_pool(name="ca_post", bufs=1) as cap:
            x2pre = cap.tile([128, WC, T], BF16, tag="x2pre", name="x2pre")
            for ob in range(WC):
                ps = mm_ps.tile([128, T], F32, tag="mm", name="ps_o2")
                gemm(ps[:, 0:T], wot2, ob, lambda ic: ocaT[:, ic, :], WC)
                nc.vector.tensor_add(x2pre[:, ob, :], ps[:, 0:T],
                                     x1T[:, ob, :])
            x2T = persist.tile([128, WC, T], BF16, tag="x2T", name="x2T")
            layernorm(lambda ic: x2pre[:, ic, :], WC, T,
                      cols['ln2_g'], cols['ln2_b'],
                      lambda ic: x2T[:, ic, :])

    # ================= FFN =================
    with nc.named_scope("ffn"), tc.tile_pool(name="ffn", bufs=1) as ffn:
        hT = ffn.tile([128, MC, T], BF16, tag="hT", name="hT")
        for oc in range(MC):
            land = ffn.tile([128, W], F32, tag="w1l", bufs=3, name="w1_land")
            nc.sync.dma_start(land[:],
                              dram['ffn_w1'][oc * 128:(oc + 1) * 128, :])
            cst = ffn.tile([128, W], BF16, tag="w1c", bufs=3, name="w1_cast")
            nc.scalar.copy(cst[:], land[:])
            w1t = ffn.tile([128, WC, 128], BF16, tag="w1t", bufs=3,
                           name="w1t")
            nc.sync.dma_start_transpose(w1t[:], cst[:])
            ps = mm_ps.tile([128, T], F32, tag="mm", name="ps_h")
            for ic in range(WC):
                nc.tensor.matmul(ps[:, 0:T], w1t[:, ic, :], x2T[:, ic, :],
                                 start=(ic == 0), stop=(ic == WC - 1))
            nc.scalar.activation(hT[:, oc, :], ps[:, 0:T], AF.Relu,
                                 bias=b1_col[:, oc:oc + 1])
        x3pre = ffn.tile([128, WC, T], BF16, tag="x3pre", name="x3pre")
        for ob in range(WC):
            w2t = ffn.tile([128, MC, 128], BF16, tag="w2t", bufs=2,
                           name="w2t")
            for piece in range(MLP // 2048):
                land = ffn.tile([128, 2048], F32, tag="w2l", bufs=3,
                                name="w2_land")
                nc.sync.dma_start(
                    land[:], dram['ffn_w2'][ob * 128:(ob + 1) * 128,
                                            piece * 2048:(piece + 1) * 2048])
                cst = ffn.tile([128, 2048], BF16, tag="w2c", bufs=3,
                               name="w2_cast")
                nc.scalar.copy(cst[:], land[:])
                nc.sync.dma_start_transpose(
                    w2t[:, piece * 16:(piece + 1) * 16, :], cst[:])
            ps = mm_ps.tile([128, T], F32, tag="mm", name="ps_f")
            for ic in range(MC):
                nc.tensor.matmul(ps[:, 0:T], w2t[:, ic, :], hT[:, ic, :],
                                 start=(ic == 0), stop=(ic == MC - 1))
            tmp = scratch.tile([128, T], F32, tag="ftmp", name="f_tmp")
            nc.scalar.activation(tmp[:, 0:T], ps[:, 0:T], AF.Identity,
                                 bias=cols['ffn_b2'][:, ob:ob + 1])
            nc.vector.tensor_add(x3pre[:, ob, :], tmp[:, 0:T], x2T[:, ob, :])
        x3T = ffn.tile([128, WC, T], F32, tag="x3T", name="x3T")
        layernorm(lambda ic: x3pre[:, ic, :], WC, T,
                  cols['ln3_g'], cols['ln3_b'],
                  lambda ic: x3T[:, ic, :])
        for tcx in range(TC):
            o_tm = ffn.tile([128, W], F32, tag="o_tm", bufs=2, name="o_tm")
            for g in range(WC // 4):
                pt = tp_ps.tile([128, 512], F32, tag="tp", name="pt_out")
                for k in range(4):
                    nc.tensor.transpose(
                        pt[:, k * 128:(k + 1) * 128],
                        x3T[:, g * 4 + k, tcx * 128:(tcx + 1) * 128],
                        ident[:])
                nc.vector.tensor_copy(o_tm[:, g * 512:(g + 1) * 512], pt[:])
            nc.sync.dma_start(out_flat[tcx * 128:(tcx + 1) * 128, :], o_tm[:])

    return out_d


_PROGRAM_CACHE = {}


def _get_program(B_loc, NQ, S, W, NH, MLP, JC=512, repeat=1):
    key = (B_loc, NQ, S, W, NH, MLP, JC, repeat)
    if key not in _PROGRAM_CACHE:
        nc = bacc.Bacc("TRN2", target_bir_lowering=False, debug=False)
        with tile.TileContext(nc) as tc, \
             nc.allow_low_precision(reason="bf16 matmul pipeline"):
            for r in range(repeat):
                with ExitStack() as ctx:
                    build_decoder(nc, tc, ctx, B_loc, NQ, S, W, NH, MLP, JC,
                                  suffix=("" if r == 0 else f"_r{r}"))
        nc.compile()
        _PROGRAM_CACHE[key] = nc
    return _PROGRAM_CACHE[key]


def _make_in_maps(inputs):
    B = inputs['query'].shape[0]
    B_loc = B // N_CORES
    shard_names = {'query', 'enc_mem', 'out_pos_enc'}
    in_maps = []
    for c in range(N_CORES):
        m = {}
        for k, v in inputs.items():
            v = np.ascontiguousarray(np.asarray(v, dtype=np.float32))
            if k in shard_names:
                m[k] = np.ascontiguousarray(v[c * B_loc:(c + 1) * B_loc])
            else:
                m[k] = v
        in_maps.append(m)
    return in_maps


def kernel(**inputs):
    B, NQ, W = inputs['query'].shape
    S = inputs['enc_mem'].shape[1]
    MLP = inputs['ffn_w1'].shape[0]
    NH = 16
    assert B % N_CORES == 0
    B_loc = B // N_CORES

    nc = _get_program(B_loc, NQ, S, W, NH, MLP)
    in_maps = _make_in_maps(inputs)

    res = run_bass_kernel_spmd(nc, in_maps, list(range(N_CORES)))
    return np.concatenate([res.results[c]["out"] for c in range(N_CORES)],
                          axis=0)
